# revision 43
# baseline (speedup 1.0000x reference)
"""Trainium2 Bass kernel for nn_DecoderCell (B=128,N=512,C=4,T=128,D=128,H=8).

Strategy: pure data-parallel over batch B across 8 NeuronCores (16 b/core).
Per batch, attention scores are computed transposed ([n, q] layout, q=(t,c))
with per-head K=16 matmuls packed 4-at-a-time onto distinct PE row groups so
they execute concurrently; the boolean mask is applied as a keep-mask (0/1)
multiply on the Vector engine after the exp (exp(s+NEG*m) == exp(s)*keep),
which keeps full-array identity matmuls off the PE. U=V@expS uses 4
concurrent column-tiled matmuls. Softmax runs unnormalized (exp on ScalarE,
denominators via an augmented-V ones column), and the final log-softmax uses
ACT accum_out row sums plus a DVE bit-twiddle ln. Matmul operands are bf16
(PSUM accumulation in f32).
"""
import numpy as np
import ml_dtypes

D = 128
N = 512
C = 4
T = 128
Q = T * C          # 512 queries per batch, q = t*C + c
H = 8
DH = 16
NB = 16            # batches per core
NCORES = 8
NEG = -1e9         # reference mask value
NEGT = -1e8        # written into tanh slots; ×10 → -1e9

# ln(m) on [1,2), power-series coeffs (deg 7, max err 5.6e-7)
LN_COEF = [
    -2.242481818575902, 4.911042808776086, -5.126667255647402,
    3.932633388234101, -2.0202020938525127, 0.6590148821953554,
    -0.12345843186141042, 0.010119082927599773,
]
LN2 = 0.6931471805599453

HA = [0, 1, 2, 3]
HB = [4, 5, 6, 7]
BF = ml_dtypes.bfloat16
F8 = ml_dtypes.float8_e4m3


def _perm_cols(W, heads):
    """Columns of W[*,128] so head g sits at cols 32g..32g+15, zeros after."""
    out = np.zeros_like(W)
    for g, h in enumerate(heads):
        out[:, 32 * g:32 * g + 16] = W[:, 16 * h:16 * h + 16]
    return out


def _perm_rows(W, heads):
    out = np.zeros_like(W)
    for g, h in enumerate(heads):
        out[32 * g:32 * g + 16, :] = W[16 * h:16 * h + 16, :]
    return out


def _host_prep(inputs):
    """Full-input numpy prep -> per-core input dicts."""
    ne = np.ascontiguousarray(inputs["node_embeddings"], np.float32)
    ge = np.ascontiguousarray(inputs["graph_embedding"], np.float32)
    sc = np.ascontiguousarray(inputs["step_context"], np.float32)
    mask = np.asarray(inputs["mask"])
    B = ne.shape[0]

    net = ne.transpose(0, 2, 1).astype(BF)                        # [B,D,N]
    scq = sc[:, :, :, 0, :].transpose(1, 3, 0, 2).reshape(B, D + 1, Q)
    scm = scq[:, :D, :].astype(BF)
    scl = scq[:, D:, :].astype(BF)
    m4 = mask[:, :, :, 0, :]                                      # [T,B,C,N] bool
    # keep mask (1 = unmasked) transposed [B, N, Q] bf16
    mkb = (1.0 - m4.transpose(1, 3, 0, 2).reshape(B, N, Q)
           .astype(np.float32)).astype(BF)
    mknat = m4.transpose(1, 0, 2, 3).reshape(B, Q, N).astype(np.uint8)
    gt = np.ascontiguousarray(ge.T).astype(BF)                    # [D,B]

    s = np.float32(1.0 / np.sqrt(DH))
    Wk1 = np.asarray(inputs["Wk1"], np.float32)
    Wqs = np.asarray(inputs["Wq_step"], np.float32) * s
    Wqf = np.asarray(inputs["Wq_fixed"], np.float32) * s
    wqsa, wqsb = _perm_cols(Wqs, HA), _perm_cols(Wqs, HB)
    # graph-embedding term precomputed on host: qfix[b] = ge[b] @ Wq_fixed*s,
    # then head-permuted; packed with the D+1-th context row's weights into a
    # rank-2 matmul rhs [scl; ones]
    qfix = ge.astype(np.float32) @ Wqf                      # [B, D]
    qfa = _perm_cols(qfix, HA)                              # [B, D] per-pass
    qfb = _perm_cols(qfix, HB)
    bfw = lambda x: np.ascontiguousarray(x).astype(BF)
    wqsal, wqsbl = wqsa[D], wqsb[D]                         # [D] rank-1 rows
    weights = {
        "wk1a": bfw(_perm_cols(Wk1, HA)), "wk1b": bfw(_perm_cols(Wk1, HB)),
        "wqsam": bfw(wqsa[:D]),
        "wqsbm": bfw(wqsb[:D]),
        "wv": bfw(inputs["Wv"]),
        "wk2s": bfw(np.asarray(inputs["Wk2"], np.float32)
                    / np.float32(np.sqrt(D))),
        "wouta": bfw(_perm_rows(np.asarray(inputs["Wout"], np.float32), HA)),
        "woutb": bfw(_perm_rows(np.asarray(inputs["Wout"], np.float32), HB)),
        "ident": np.eye(128, dtype=np.float32),
        # e4[g, 32g+r] = 1 for r<16 (normalization broadcast)
        "e4": np.stack([
            np.concatenate([np.zeros(32 * g, np.float32),
                            np.ones(16, np.float32),
                            np.zeros(128 - 32 * g - 16, np.float32)])
            for g in range(4)]).astype(BF),
        # g16[32g+16, g] = 1 (Z row gather)
        "g16": np.stack([
            (np.arange(128) == 32 * g + 16).astype(np.float32)
            for g in range(4)], axis=1).astype(BF),
        # p432[tp, 4tp+c] = 1 (lnZ partition broadcast)
        "p432": np.stack([
            ((np.arange(128) // 4) == tp).astype(np.float32)
            for tp in range(32)]),
    }

    core_ins = []
    for ci in range(NCORES):
        b0 = ci * NB
        sl = slice(b0, b0 + NB)
        m = dict(weights)
        m.update({
            "net": np.ascontiguousarray(net[sl]),
            "scm": np.ascontiguousarray(scm[sl]),
            "scl2": np.ascontiguousarray(np.stack(
                [scl[sl][:, 0, :], np.ones_like(scl[sl][:, 0, :])],
                axis=1)).astype(BF),
            "qf2a": np.ascontiguousarray(np.stack(
                [np.broadcast_to(wqsal, (NB, D)),
                 qfa[sl]], axis=1)).astype(BF),
            "qf2b": np.ascontiguousarray(np.stack(
                [np.broadcast_to(wqsbl, (NB, D)),
                 qfb[sl]], axis=1)).astype(BF),
            "mkb": np.ascontiguousarray(mkb[sl]),
            "mknat": np.ascontiguousarray(mknat[sl]),
        })
        core_ins.append(m)
    return core_ins


def build_kernel(nb=NB):
    import concourse.bacc as bacc
    import concourse.mybir as mybir
    import concourse.tile as tile

    dt = mybir.dt
    f32, bf16, u8, i32 = dt.float32, dt.bfloat16, dt.uint8, dt.int32
    AF = mybir.ActivationFunctionType
    OP = mybir.AluOpType

    nc = bacc.Bacc("TRN2", target_bir_lowering=False, debug=False,
                   num_devices=NCORES)

    din = {}
    def dram(name, shape, dtype, kind="ExternalInput"):
        din[name] = nc.dram_tensor(name, shape, dtype, kind=kind)
        return din[name]

    net = dram("net", [nb, D, N], bf16)
    scm = dram("scm", [nb, D, Q], bf16)
    scl2 = dram("scl2", [nb, 2, Q], bf16)
    qf2a = dram("qf2a", [nb, 2, D], bf16)
    qf2b = dram("qf2b", [nb, 2, D], bf16)
    mkb = dram("mkb", [nb, N, Q], bf16)
    mknat = dram("mknat", [nb, Q, N], u8)
    for w in ("wk1a", "wk1b", "wqsam", "wqsbm", "wv", "wk2s",
              "wouta", "woutb", "e4", "g16"):
        shape = [4, 128] if w == "e4" else ([128, 4] if w == "g16" else [128, 128])
        dram(w, shape, bf16)
    dram("ident", [128, 128], f32)
    dram("p432", [32, 128], f32)
    # device layout [q'=(t', c), b, i, n]; host reassembles t = 32*i + t'
    out = dram("out", [128, nb, 4, N], f32, kind="ExternalOutput")

    with tile.TileContext(nc) as tc:
        from contextlib import ExitStack
        with ExitStack() as ctx:
            wp = ctx.enter_context(tc.tile_pool(name="wp", bufs=1))
            io = ctx.enter_context(tc.tile_pool(name="io", bufs=2))
            wk = ctx.enter_context(tc.tile_pool(name="wk", bufs=2))
            esp = ctx.enter_context(tc.tile_pool(name="esp", bufs=4))
            big = ctx.enter_context(tc.tile_pool(name="big", bufs=2))
            sm = ctx.enter_context(tc.tile_pool(name="sm", bufs=2))
            # PSUM: 8 banks total.
            # pscore 2x[128,2,512]=4, pacc 1x[128,512]=1, pmisc 2x[128,512]=2,
            # ptiny 1x[*,<=512]=1
            pscore = ctx.enter_context(
                tc.tile_pool(name="pscore", bufs=2, space="PSUM"))
            pacc = ctx.enter_context(
                tc.tile_pool(name="pacc", bufs=1, space="PSUM"))
            pmisc = ctx.enter_context(
                tc.tile_pool(name="pmisc", bufs=2, space="PSUM"))
            ptiny = ctx.enter_context(
                tc.tile_pool(name="ptiny", bufs=1, space="PSUM"))

            # --- static weights/constants to SBUF ---
            # prologue-critical weights first; the rest queue after batch 0's
            # input DMAs so the first projections start sooner
            W = {}
            def load_w(names):
                for wn in names:
                    t = wp.tile(list(din[wn].shape), din[wn].dtype,
                                tag=f"w_{wn}", name=f"w_{wn}")
                    nc.sync.dma_start(out=t, in_=din[wn][:, :])
                    W[wn] = t
            load_w(("wk1a", "wk1b", "wk2s", "wv", "wqsam", "wqsbm"))
            negt = wp.tile([128, C, N], f32, tag="negt")
            nc.gpsimd.memset(negt, NEGT)

            pend_epi = []
            pend_pro = []

            def drain_epi(n=1):
                for _ in range(min(n, len(pend_epi))):
                    pend_epi.pop(0)()

            def drain_pro(n=1):
                for _ in range(min(n, len(pend_pro))):
                    pend_pro.pop(0)()

            def build_prologue(b, dst):
                """Closure list loading batch b's inputs and computing its
                projections into `dst`; drained during batch b-1's attention
                so the serial projection chain overlaps useful work."""

                def p_dma():
                    net_t = io.tile([D, N], bf16, tag="net", name="net_t")
                    nc.sync.dma_start(out=net_t, in_=net[b])
                    scm_t = io.tile([D, Q], bf16, tag="scm", name="scm_t")
                    nc.sync.dma_start(out=scm_t, in_=scm[b])
                    scl_t = io.tile([2, Q], bf16, tag="scl", name="scl_t")
                    nc.sync.dma_start(out=scl_t, in_=scl2[b])
                    qfa_t = io.tile([2, D], bf16, tag="qfa", name="qfa_t")
                    nc.sync.dma_start(out=qfa_t, in_=qf2a[b])
                    qfb_t = io.tile([2, D], bf16, tag="qfb", name="qfb_t")
                    nc.sync.dma_start(out=qfb_t, in_=qf2b[b])
                    dst["net"], dst["scm"], dst["scl"] = net_t, scm_t, scl_t
                    dst["qfa"], dst["qfb"] = qfa_t, qfb_t

                def p_dma_masks():
                    mkb_t = io.tile([128, 4, Q], bf16, tag="mkb",
                                    name="mkb_t")
                    nc.sync.dma_start(
                        out=mkb_t,
                        in_=mkb[b].rearrange("(j p) q -> p j q", p=128))
                    mknat_t = io.tile([128, 4, N], u8, tag="mknat",
                                      name="mknat_t")
                    nc.sync.dma_start(
                        out=mknat_t,
                        in_=mknat[b].rearrange("(i p) n -> p i n", p=128))
                    dst["mkb"], dst["mknat"] = mkb_t, mknat_t

                def p_proj(wn, tag, eng):
                    ps = pmisc.tile([128, N], f32, tag="proj", name="ps")
                    nc.tensor.matmul(ps, lhsT=W[wn], rhs=dst["net"])
                    sb = wk.tile([128, N], bf16, tag=tag, name=tag)
                    if eng == "s":
                        nc.scalar.copy(sb, ps)
                    else:
                        nc.vector.tensor_copy(sb, ps)
                    dst[tag] = sb

                def p_vaug_init():
                    vauga = wk.tile([128, 4, 128], bf16, tag="vauga",
                                    name="vauga")
                    vaugb = wk.tile([128, 4, 128], bf16, tag="vaugb",
                                    name="vaugb")
                    for va in (vauga, vaugb):
                        nc.gpsimd.memset(va, 0.0)
                        nc.gpsimd.memset(
                            va.rearrange("p j (g r) -> p j g r",
                                         g=4)[:, :, :, 16:17], 1.0)
                    dst["vauga"], dst["vaugb"] = vauga, vaugb

                def p_v(j):
                    pv = pmisc.tile([128, 128], f32, tag="proj", name="pv")
                    nc.tensor.matmul(
                        pv[:, :128], lhsT=dst["net"][:, 128 * j:128 * (j + 1)],
                        rhs=W["wv"])
                    for va, c0 in ((dst["vauga"], 0), (dst["vaugb"], 64)):
                        nc.vector.tensor_copy(
                            va[:, j, :].rearrange("p (g r) -> p g r",
                                                  g=4)[:, :, 0:16],
                            pv[:, c0:c0 + 64].rearrange("p (g r) -> p g r",
                                                        g=4))

                def p_q1(wm, qf, tag):
                    ps = pmisc.tile([128, Q], f32, tag="proj", name="ps")
                    nc.tensor.matmul(ps, lhsT=W[wm], rhs=dst["scm"],
                                     start=True, stop=False)
                    nc.tensor.matmul(ps, lhsT=dst[qf], rhs=dst["scl"],
                                     start=False, stop=True)
                    sb = wk.tile([128, Q], bf16, tag=tag, name=tag)
                    nc.scalar.copy(sb, ps)
                    dst[tag] = sb

                return [
                    p_dma,
                    p_dma_masks,
                    lambda: p_proj("wk1a", "k1ta", "v"),
                    lambda: p_proj("wk1b", "k1tb", "v"),
                    lambda: p_proj("wk2s", "k2t", "s"),
                    p_vaug_init,
                    lambda: p_v(0), lambda: p_v(1),
                    lambda: p_v(2), lambda: p_v(3),
                    lambda: p_q1("wqsam", "qfa", "q1ta"),
                    lambda: p_q1("wqsbm", "qfb", "q1tb"),
                ]

            def build_epilogue(b, usb, zsb, k2t, mknat_t):
                """Closure list for batch b's post-attention chain; emitted
                interleaved into batch b+1's attention to avoid head-of-line
                stalls on the in-order engine queues."""
                st = {}

                def s_recip():
                    rinv = sm.tile([4, 1024], f32, tag="rinv", name="rinv")
                    nc.vector.reciprocal_approx_fast(out=rinv, in_=zsb)
                    rinvb = sm.tile([4, 1024], bf16, tag="rinvb", name="rinvb")
                    nc.vector.tensor_copy(rinvb, rinv)
                    st["rinvb"] = rinvb

                def s_un(pi):
                    pbc = pmisc.tile([128, Q], f32, tag="proj", name="pbc")
                    nc.tensor.matmul(
                        pbc, lhsT=W["e4"],
                        rhs=st["rinvb"][:, Q * pi:Q * (pi + 1)])
                    u_n = wk.tile([128, Q], bf16, tag=f"un{pi}", name="u_n")
                    nc.vector.tensor_tensor(u_n, usb[pi], pbc, OP.mult)
                    st[f"un{pi}"] = u_n

                def s_q2():
                    pq2 = pmisc.tile([128, Q], f32, tag="proj", name="pq2")
                    nc.tensor.matmul(pq2, lhsT=W["wouta"], rhs=st["un0"],
                                     start=True, stop=False)
                    nc.tensor.matmul(pq2, lhsT=W["woutb"], rhs=st["un1"],
                                     start=False, stop=True)
                    q2t = wk.tile([128, Q], bf16, tag="q2t", name="q2t")
                    nc.vector.tensor_copy(q2t, pq2)
                    st["q2t"] = q2t

                def s_tanh(ii):
                    if ii == 0:
                        st["tanh_sb"] = big.tile([128, C, N], f32, tag="tanh",
                                                 name="tanh_sb")
                    pl = pscore.tile([128, 2, N], f32, tag="score", name="pl")
                    for i2 in range(2):
                        i = 2 * ii + i2
                        nc.tensor.matmul(
                            pl[:, i2, :],
                            lhsT=st["q2t"][:, 128 * i:128 * (i + 1)], rhs=k2t)
                    nc.scalar.activation(
                        st["tanh_sb"][:, 2 * ii:2 * ii + 2, :], pl, AF.Tanh)

                def s_pred():
                    nc.vector.copy_predicated(st["tanh_sb"], mknat_t, negt)
                    st["sacc"] = sm.tile([128, 4], f32, tag="sacc",
                                         name="sacc")
                    st["scratch"] = sm.tile([128, N], f32, tag="scratch",
                                            name="scratch")

                def s_exp(i):
                    nc.scalar.activation(
                        st["scratch"], st["tanh_sb"][:, i, :], AF.Exp,
                        scale=10.0, accum_out=st["sacc"][:, i:i + 1])

                def s_z2():
                    pst = ptiny.tile([4, 128], f32, tag="tiny", name="pst")
                    nc.tensor.transpose(pst, st["sacc"], W["ident"])
                    ssb = sm.tile([4, 128], f32, tag="ssb", name="ssb")
                    nc.vector.tensor_copy(ssb, pst)
                    z2 = sm.tile([4, 32], f32, tag="z2", name="z2")
                    nc.vector.tensor_reduce(
                        z2, ssb.rearrange("p (t c) -> p t c", c=4),
                        axis=mybir.AxisListType.X, op=OP.add)
                    st["z2"] = z2

                def s_ln1():
                    zi = st["z2"].bitcast(i32)
                    ei = sm.tile([4, 32], i32, tag="ei", name="ei")
                    nc.vector.tensor_scalar(ei, zi, 23, None,
                                            OP.logical_shift_right)
                    ef = sm.tile([4, 32], f32, tag="ef", name="ef")
                    nc.vector.tensor_copy(ef, ei)
                    mi = sm.tile([4, 32], i32, tag="mi", name="mi")
                    nc.vector.tensor_scalar(mi, zi, 0x7FFFFF, 0x3F800000,
                                            OP.bitwise_and, OP.bitwise_or)
                    acc = sm.tile([4, 32], f32, tag="lnacc", name="acc")
                    nc.vector.tensor_scalar(acc, mi.bitcast(f32), LN_COEF[7],
                                            LN_COEF[6], OP.mult, OP.add)
                    st["ef"], st["mi"], st["acc"] = ef, mi, acc

                def s_ln2(ks):
                    mf = st["mi"].bitcast(f32)
                    for k in ks:
                        nc.vector.tensor_tensor(st["acc"], st["acc"], mf,
                                                OP.mult)
                        nc.vector.tensor_scalar_add(st["acc"], st["acc"],
                                                    LN_COEF[k])

                def s_ln3():
                    nc.vector.tensor_scalar(st["ef"], st["ef"], LN2,
                                            -127.0 * LN2, OP.mult, OP.add)
                    nc.vector.tensor_tensor(st["acc"], st["acc"], st["ef"],
                                            OP.add)

                def s_bias():
                    pzt = ptiny.tile([32, 4], f32, tag="tiny", name="pzt")
                    nc.tensor.transpose(pzt, st["acc"], W["ident"][:4, :4])
                    lzt = sm.tile([32, 4], f32, tag="lzt", name="lzt")
                    nc.vector.tensor_copy(lzt, pzt)
                    pbias = ptiny.tile([128, 4], f32, tag="tiny", name="pbias")
                    nc.tensor.matmul(pbias, lhsT=W["p432"], rhs=lzt)
                    bias = sm.tile([128, 4], f32, tag="bias", name="bias")
                    nc.vector.tensor_copy(bias, pbias)
                    st["bias"] = bias

                def s_out(i):
                    if i == 0:
                        st["out_sb"] = big.tile([128, C, N], f32, tag="outsb",
                                                name="out_sb")
                    nc.vector.tensor_scalar(
                        st["out_sb"][:, i, :], st["tanh_sb"][:, i, :], 10.0,
                        st["bias"][:, i:i + 1], OP.mult, OP.subtract)
                    if i == 3:
                        nc.sync.dma_start(out=out[:, b, :, :],
                                          in_=st["out_sb"])

                return [
                    s_recip,
                    lambda: s_un(0), lambda: s_un(1),
                    s_q2,
                    lambda: s_tanh(0), lambda: s_tanh(1),
                    s_pred,
                    lambda: s_exp(0), lambda: s_exp(1),
                    lambda: s_exp(2), lambda: s_exp(3),
                    s_z2, s_ln1,
                    lambda: s_ln2([5, 4, 3]), lambda: s_ln2([2, 1, 0]),
                    s_ln3, s_bias,
                    lambda: s_out(0), lambda: s_out(1),
                    lambda: s_out(2), lambda: s_out(3),
                ]

            # prologue of batch 0 runs up front
            cur = {}
            pro0 = build_prologue(0, cur)
            pro0[0]()           # batch 0 input DMAs queue first
            load_w(("wouta", "woutb", "ident", "e4", "g16", "p432"))
            for step in pro0[1:]:
                step()

            for b in range(nb):
                nxt = {}
                if b + 1 < nb:
                    pend_pro = build_prologue(b + 1, nxt)

                # ---------- attention passes ----------
                zsb = sm.tile([4, 1024], f32, tag="zsb")
                usb = {}
                psu = {}
                vaugs = (cur["vauga"], cur["vaugb"])
                mkb_t = cur["mkb"]

                def issue_u(pi, j, esA, esB):
                    if j == 0:
                        psu[pi] = pacc.tile([128, Q], f32, tag="u",
                                            name=f"psu{pi}")
                    # 4 concurrent col-tiled U MMs (accumulate over j)
                    for g in range(4):
                        es = esA if g < 2 else esB
                        nc.tensor.matmul(
                            psu[pi][32 * g:32 * g + 32, :],
                            lhsT=vaugs[pi][:, j, 32 * g:32 * g + 32],
                            rhs=es[:, g % 2, :],
                            start=(j == 0), stop=(j == 3),
                            tile_position=(0, 32 * g),
                            skip_group_check=True)
                    if j == 3:
                        u_sb = wk.tile([128, Q], bf16, tag="usb")
                        nc.vector.tensor_copy(u_sb, psu[pi])
                        usb[pi] = u_sb
                        pz = ptiny.tile([4, Q], f32, tag="tiny")
                        nc.tensor.matmul(pz, lhsT=W["g16"], rhs=u_sb)
                        nc.vector.tensor_copy(zsb[:, Q * pi:Q * (pi + 1)], pz)

                prev = None
                for pi, (k1t, q1t_sb) in enumerate(
                        ((cur["k1ta"], cur["q1ta"]), (cur["k1tb"], cur["q1tb"]))):
                    for j in range(4):
                        # 4 concurrent row-tiled score MMs (strips 0..3)
                        pssA = pscore.tile([128, 2, Q], f32, tag="score")
                        pssB = pscore.tile([128, 2, Q], f32, tag="score")
                        for g in range(4):
                            pss = pssA if g < 2 else pssB
                            # K=32 (rows 16..31 of each strip are zero by the
                            # head-perm layout) - full 32-row tiles overlap
                            # better than K=16 slices
                            sl = slice(32 * g, 32 * g + 32)
                            nc.tensor.matmul(
                                pss[:, g % 2, :],
                                lhsT=k1t[sl, 128 * j:128 * (j + 1)],
                                rhs=q1t_sb[sl, :],
                                tile_position=(32 * g, 0))
                        # exp -> bf16, then keep-mask multiply on DVE
                        esA = esp.tile([128, 2, Q], bf16, tag="esA")
                        esB = esp.tile([128, 2, Q], bf16, tag="esB")
                        mk_b = mkb_t[:, j:j + 1, :].broadcast_to([128, 2, Q])
                        nc.scalar.activation(esA, pssA, AF.Exp)
                        nc.vector.tensor_tensor(esA, esA, mk_b, OP.mult)
                        nc.scalar.activation(esB, pssB, AF.Exp)
                        nc.vector.tensor_tensor(esB, esB, mk_b, OP.mult)
                        if prev is not None:
                            issue_u(*prev)
                        prev = (pi, j, esA, esB)
                        drain_epi(2)
                        drain_pro(1)
                issue_u(*prev)
                drain_epi(99)
                drain_pro(99)

                pend_epi = build_epilogue(b, usb, zsb, cur["k2t"],
                                          cur["mknat"])
                cur = nxt

            while pend_epi:
                pend_epi.pop(0)()

    nc.compile()
    return nc


_CACHED = None


def _get_nc():
    global _CACHED
    if _CACHED is None:
        _CACHED = build_kernel()
    return _CACHED


def kernel(**inputs):
    from concourse.bass_utils import run_bass_kernel_spmd

    core_ins = _host_prep(inputs)
    nc = _get_nc()
    res = run_bass_kernel_spmd(nc, core_ins, core_ids=list(range(NCORES)))
    outs = [_unscramble(r["out"]) for r in res.results]   # each [T, NB, 2048]
    return np.concatenate(outs, axis=1)                   # [T, B, 2048]


def _unscramble(dev):
    """Device [128 q'=(t',c), nb, 4 i, 512 n] -> [T, nb, C*N] with t=32i+t'."""
    nb = dev.shape[1]
    return (dev.reshape(32, C, nb, 4, N)
            .transpose(3, 0, 2, 1, 4)
            .reshape(T, nb, C * N))


# revision 44
# speedup vs baseline: 1.0363x; 1.0363x over previous
"""Trainium2 Bass kernel for nn_DecoderCell (B=128,N=512,C=4,T=128,D=128,H=8).

Strategy: pure data-parallel over batch B across 8 NeuronCores (16 b/core).
Per batch, attention scores are computed transposed ([n, q] layout, q=(t,c))
with per-head K=16 matmuls packed 4-at-a-time onto distinct PE row groups so
they execute concurrently; the boolean mask is applied as a keep-mask (0/1)
multiply on the Vector engine after the exp (exp(s+NEG*m) == exp(s)*keep),
which keeps full-array identity matmuls off the PE. U=V@expS uses 4
concurrent column-tiled matmuls. Softmax runs unnormalized (exp on ScalarE,
denominators via an augmented-V ones column), and the final log-softmax uses
ACT accum_out row sums plus a DVE bit-twiddle ln. Matmul operands are bf16
(PSUM accumulation in f32).
"""
import numpy as np
import ml_dtypes

D = 128
N = 512
C = 4
T = 128
Q = T * C          # 512 queries per batch, q = t*C + c
H = 8
DH = 16
NB = 16            # batches per core
NCORES = 8
NEG = -1e9         # reference mask value
NEGT = -1e8        # written into tanh slots; ×10 → -1e9

# ln(m) on [1,2), power-series coeffs (deg 7, max err 5.6e-7)
LN_COEF = [
    -2.242481818575902, 4.911042808776086, -5.126667255647402,
    3.932633388234101, -2.0202020938525127, 0.6590148821953554,
    -0.12345843186141042, 0.010119082927599773,
]
LN2 = 0.6931471805599453

HA = [0, 1, 2, 3]
HB = [4, 5, 6, 7]
BF = ml_dtypes.bfloat16
F8 = ml_dtypes.float8_e4m3


def _perm_cols(W, heads):
    """Columns of W[*,128] so head g sits at cols 32g..32g+15, zeros after."""
    out = np.zeros_like(W)
    for g, h in enumerate(heads):
        out[:, 32 * g:32 * g + 16] = W[:, 16 * h:16 * h + 16]
    return out


def _perm_rows(W, heads):
    out = np.zeros_like(W)
    for g, h in enumerate(heads):
        out[32 * g:32 * g + 16, :] = W[16 * h:16 * h + 16, :]
    return out


def _host_prep(inputs):
    """Full-input numpy prep -> per-core input dicts."""
    ne = np.ascontiguousarray(inputs["node_embeddings"], np.float32)
    ge = np.ascontiguousarray(inputs["graph_embedding"], np.float32)
    sc = np.ascontiguousarray(inputs["step_context"], np.float32)
    mask = np.asarray(inputs["mask"])
    B = ne.shape[0]

    net = ne.transpose(0, 2, 1).astype(BF)                        # [B,D,N]
    scq = sc[:, :, :, 0, :].transpose(1, 3, 0, 2).reshape(B, D + 1, Q)
    scm = scq[:, :D, :].astype(BF)
    scl = scq[:, D:, :].astype(BF)
    m4 = mask[:, :, :, 0, :]                                      # [T,B,C,N] bool
    # keep mask (1 = unmasked) transposed [B, N, Q] bf16
    mkb = (1.0 - m4.transpose(1, 3, 0, 2).reshape(B, N, Q)
           .astype(np.float32)).astype(BF)
    mknat = m4.transpose(1, 0, 2, 3).reshape(B, Q, N).astype(np.uint8)
    gt = np.ascontiguousarray(ge.T).astype(BF)                    # [D,B]

    s = np.float32(1.0 / np.sqrt(DH))
    Wk1 = np.asarray(inputs["Wk1"], np.float32)
    Wqs = np.asarray(inputs["Wq_step"], np.float32) * s
    Wqf = np.asarray(inputs["Wq_fixed"], np.float32) * s
    wqsa, wqsb = _perm_cols(Wqs, HA), _perm_cols(Wqs, HB)
    # graph-embedding term precomputed on host: qfix[b] = ge[b] @ Wq_fixed*s,
    # then head-permuted; packed with the D+1-th context row's weights into a
    # rank-2 matmul rhs [scl; ones]
    qfix = ge.astype(np.float32) @ Wqf                      # [B, D]
    qfa = _perm_cols(qfix, HA)                              # [B, D] per-pass
    qfb = _perm_cols(qfix, HB)
    bfw = lambda x: np.ascontiguousarray(x).astype(BF)
    wqsal, wqsbl = wqsa[D], wqsb[D]                         # [D] rank-1 rows
    weights = {
        "wk1a": bfw(_perm_cols(Wk1, HA)), "wk1b": bfw(_perm_cols(Wk1, HB)),
        "wqsam": bfw(wqsa[:D]),
        "wqsbm": bfw(wqsb[:D]),
        "wv": bfw(inputs["Wv"]),
        "wk2s": bfw(np.asarray(inputs["Wk2"], np.float32)
                    / np.float32(np.sqrt(D))),
        "wouta": bfw(_perm_rows(np.asarray(inputs["Wout"], np.float32), HA)),
        "woutb": bfw(_perm_rows(np.asarray(inputs["Wout"], np.float32), HB)),
        "ident": np.eye(128, dtype=np.float32),
        # e4[g, 32g+r] = 1 for r<16 (normalization broadcast)
        "e4": np.stack([
            np.concatenate([np.zeros(32 * g, np.float32),
                            np.ones(16, np.float32),
                            np.zeros(128 - 32 * g - 16, np.float32)])
            for g in range(4)]).astype(BF),
        # g16[32g+16, g] = 1 (Z row gather)
        "g16": np.stack([
            (np.arange(128) == 32 * g + 16).astype(np.float32)
            for g in range(4)], axis=1).astype(BF),
        # p432[tp, 4tp+c] = 1 (lnZ partition broadcast)
        "p432": np.stack([
            ((np.arange(128) // 4) == tp).astype(np.float32)
            for tp in range(32)]),
    }

    core_ins = []
    for ci in range(NCORES):
        b0 = ci * NB
        sl = slice(b0, b0 + NB)
        m = dict(weights)
        m.update({
            "net": np.ascontiguousarray(net[sl]),
            "scm": np.ascontiguousarray(scm[sl]),
            "scl2": np.ascontiguousarray(np.stack(
                [scl[sl][:, 0, :], np.ones_like(scl[sl][:, 0, :])],
                axis=1)).astype(BF),
            "qf2a": np.ascontiguousarray(np.stack(
                [np.broadcast_to(wqsal, (NB, D)),
                 qfa[sl]], axis=1)).astype(BF),
            "qf2b": np.ascontiguousarray(np.stack(
                [np.broadcast_to(wqsbl, (NB, D)),
                 qfb[sl]], axis=1)).astype(BF),
            "mkb": np.ascontiguousarray(mkb[sl]),
            "mknat": np.ascontiguousarray(mknat[sl]),
        })
        core_ins.append(m)
    return core_ins


def build_kernel(nb=NB):
    import concourse.bacc as bacc
    import concourse.mybir as mybir
    import concourse.tile as tile

    dt = mybir.dt
    f32, bf16, u8, i32 = dt.float32, dt.bfloat16, dt.uint8, dt.int32
    AF = mybir.ActivationFunctionType
    OP = mybir.AluOpType

    nc = bacc.Bacc("TRN2", target_bir_lowering=False, debug=False,
                   num_devices=NCORES)

    din = {}
    def dram(name, shape, dtype, kind="ExternalInput"):
        din[name] = nc.dram_tensor(name, shape, dtype, kind=kind)
        return din[name]

    net = dram("net", [nb, D, N], bf16)
    scm = dram("scm", [nb, D, Q], bf16)
    scl2 = dram("scl2", [nb, 2, Q], bf16)
    qf2a = dram("qf2a", [nb, 2, D], bf16)
    qf2b = dram("qf2b", [nb, 2, D], bf16)
    mkb = dram("mkb", [nb, N, Q], bf16)
    mknat = dram("mknat", [nb, Q, N], u8)
    for w in ("wk1a", "wk1b", "wqsam", "wqsbm", "wv", "wk2s",
              "wouta", "woutb", "e4", "g16"):
        shape = [4, 128] if w == "e4" else ([128, 4] if w == "g16" else [128, 128])
        dram(w, shape, bf16)
    dram("ident", [128, 128], f32)
    dram("p432", [32, 128], f32)
    # device layout [q'=(t', c), b, i, n]; host reassembles t = 32*i + t'
    out = dram("out", [128, nb, 4, N], f32, kind="ExternalOutput")

    with tile.TileContext(nc) as tc:
        from contextlib import ExitStack
        with ExitStack() as ctx:
            wp = ctx.enter_context(tc.tile_pool(name="wp", bufs=1))
            io = ctx.enter_context(tc.tile_pool(name="io", bufs=2))
            wk = ctx.enter_context(tc.tile_pool(name="wk", bufs=2))
            esp = ctx.enter_context(tc.tile_pool(name="esp", bufs=4))
            big = ctx.enter_context(tc.tile_pool(name="big", bufs=2))
            sm = ctx.enter_context(tc.tile_pool(name="sm", bufs=2))
            # PSUM: 8 banks total.
            # pscore 2x[128,2,512]=4, pacc 1x[128,512]=1, pmisc 2x[128,512]=2,
            # ptiny 1x[*,<=512]=1
            pscore = ctx.enter_context(
                tc.tile_pool(name="pscore", bufs=2, space="PSUM"))
            pacc = ctx.enter_context(
                tc.tile_pool(name="pacc", bufs=1, space="PSUM"))
            pmisc = ctx.enter_context(
                tc.tile_pool(name="pmisc", bufs=2, space="PSUM"))
            ptiny = ctx.enter_context(
                tc.tile_pool(name="ptiny", bufs=1, space="PSUM"))

            # --- static weights/constants to SBUF ---
            W = {}
            for wn in ("wk1a", "wk1b", "wqsam", "wqsbm", "wv",
                       "wk2s", "wouta", "woutb", "ident", "e4", "g16",
                       "p432"):
                t = wp.tile(list(din[wn].shape), din[wn].dtype, tag=f"w_{wn}")
                nc.sync.dma_start(out=t, in_=din[wn][:, :])
                W[wn] = t
            negt = wp.tile([128, C, N], f32, tag="negt")
            nc.gpsimd.memset(negt, NEGT)

            pend_epi = []
            pend_pro = []

            def drain_epi(n=1):
                for _ in range(min(n, len(pend_epi))):
                    pend_epi.pop(0)()

            def drain_pro(n=1):
                for _ in range(min(n, len(pend_pro))):
                    pend_pro.pop(0)()

            def build_prologue(b, dst):
                """Closure list loading batch b's inputs and computing its
                projections into `dst`; drained during batch b-1's attention
                so the serial projection chain overlaps useful work."""

                def p_dma():
                    net_t = io.tile([D, N], bf16, tag="net", name="net_t")
                    nc.sync.dma_start(out=net_t, in_=net[b])
                    scm_t = io.tile([D, Q], bf16, tag="scm", name="scm_t")
                    nc.sync.dma_start(out=scm_t, in_=scm[b])
                    scl_t = io.tile([2, Q], bf16, tag="scl", name="scl_t")
                    nc.sync.dma_start(out=scl_t, in_=scl2[b])
                    qfa_t = io.tile([2, D], bf16, tag="qfa", name="qfa_t")
                    nc.sync.dma_start(out=qfa_t, in_=qf2a[b])
                    qfb_t = io.tile([2, D], bf16, tag="qfb", name="qfb_t")
                    nc.sync.dma_start(out=qfb_t, in_=qf2b[b])
                    mkb_t = io.tile([128, 4, Q], bf16, tag="mkb",
                                    name="mkb_t")
                    nc.sync.dma_start(
                        out=mkb_t,
                        in_=mkb[b].rearrange("(j p) q -> p j q", p=128))
                    mknat_t = io.tile([128, 4, N], u8, tag="mknat",
                                      name="mknat_t")
                    nc.sync.dma_start(
                        out=mknat_t,
                        in_=mknat[b].rearrange("(i p) n -> p i n", p=128))
                    dst["net"], dst["scm"], dst["scl"] = net_t, scm_t, scl_t
                    dst["qfa"], dst["qfb"] = qfa_t, qfb_t
                    dst["mkb"], dst["mknat"] = mkb_t, mknat_t

                def p_proj(wn, tag, eng):
                    ps = pmisc.tile([128, N], f32, tag="proj", name="ps")
                    nc.tensor.matmul(ps, lhsT=W[wn], rhs=dst["net"])
                    sb = wk.tile([128, N], bf16, tag=tag, name=tag)
                    if eng == "s":
                        nc.scalar.copy(sb, ps)
                    else:
                        nc.vector.tensor_copy(sb, ps)
                    dst[tag] = sb

                def p_vaug_init():
                    vauga = wk.tile([128, 4, 128], bf16, tag="vauga",
                                    name="vauga")
                    vaugb = wk.tile([128, 4, 128], bf16, tag="vaugb",
                                    name="vaugb")
                    for va in (vauga, vaugb):
                        nc.gpsimd.memset(va, 0.0)
                        nc.gpsimd.memset(
                            va.rearrange("p j (g r) -> p j g r",
                                         g=4)[:, :, :, 16:17], 1.0)
                    dst["vauga"], dst["vaugb"] = vauga, vaugb

                def p_v(j):
                    pv = pmisc.tile([128, 128], f32, tag="proj", name="pv")
                    nc.tensor.matmul(
                        pv[:, :128], lhsT=dst["net"][:, 128 * j:128 * (j + 1)],
                        rhs=W["wv"])
                    for va, c0 in ((dst["vauga"], 0), (dst["vaugb"], 64)):
                        nc.vector.tensor_copy(
                            va[:, j, :].rearrange("p (g r) -> p g r",
                                                  g=4)[:, :, 0:16],
                            pv[:, c0:c0 + 64].rearrange("p (g r) -> p g r",
                                                        g=4))

                def p_q1(wm, qf, tag):
                    ps = pmisc.tile([128, Q], f32, tag="proj", name="ps")
                    nc.tensor.matmul(ps, lhsT=W[wm], rhs=dst["scm"],
                                     start=True, stop=False)
                    nc.tensor.matmul(ps, lhsT=dst[qf], rhs=dst["scl"],
                                     start=False, stop=True)
                    sb = wk.tile([128, Q], bf16, tag=tag, name=tag)
                    nc.scalar.copy(sb, ps)
                    dst[tag] = sb

                return [
                    p_dma,
                    lambda: p_proj("wk1a", "k1ta", "v"),
                    lambda: p_proj("wk1b", "k1tb", "v"),
                    lambda: p_proj("wk2s", "k2t", "s"),
                    p_vaug_init,
                    lambda: p_v(0), lambda: p_v(1),
                    lambda: p_v(2), lambda: p_v(3),
                    lambda: p_q1("wqsam", "qfa", "q1ta"),
                    lambda: p_q1("wqsbm", "qfb", "q1tb"),
                ]

            def build_epilogue(b, usb, zsb, k2t, mknat_t):
                """Closure list for batch b's post-attention chain; emitted
                interleaved into batch b+1's attention to avoid head-of-line
                stalls on the in-order engine queues."""
                st = {}

                def s_recip():
                    rinv = sm.tile([4, 1024], f32, tag="rinv", name="rinv")
                    nc.vector.reciprocal_approx_fast(out=rinv, in_=zsb)
                    rinvb = sm.tile([4, 1024], bf16, tag="rinvb", name="rinvb")
                    nc.vector.tensor_copy(rinvb, rinv)
                    st["rinvb"] = rinvb

                def s_un(pi):
                    pbc = pmisc.tile([128, Q], f32, tag="proj", name="pbc")
                    nc.tensor.matmul(
                        pbc, lhsT=W["e4"],
                        rhs=st["rinvb"][:, Q * pi:Q * (pi + 1)])
                    u_n = wk.tile([128, Q], bf16, tag=f"un{pi}", name="u_n")
                    nc.vector.tensor_tensor(u_n, usb[pi], pbc, OP.mult)
                    st[f"un{pi}"] = u_n

                def s_q2():
                    pq2 = pmisc.tile([128, Q], f32, tag="proj", name="pq2")
                    nc.tensor.matmul(pq2, lhsT=W["wouta"], rhs=st["un0"],
                                     start=True, stop=False)
                    nc.tensor.matmul(pq2, lhsT=W["woutb"], rhs=st["un1"],
                                     start=False, stop=True)
                    q2t = wk.tile([128, Q], bf16, tag="q2t", name="q2t")
                    nc.vector.tensor_copy(q2t, pq2)
                    st["q2t"] = q2t

                def s_tanh(ii):
                    if ii == 0:
                        st["tanh_sb"] = big.tile([128, C, N], f32, tag="tanh",
                                                 name="tanh_sb")
                    pl = pscore.tile([128, 2, N], f32, tag="score", name="pl")
                    for i2 in range(2):
                        i = 2 * ii + i2
                        nc.tensor.matmul(
                            pl[:, i2, :],
                            lhsT=st["q2t"][:, 128 * i:128 * (i + 1)], rhs=k2t)
                    nc.scalar.activation(
                        st["tanh_sb"][:, 2 * ii:2 * ii + 2, :], pl, AF.Tanh)

                def s_pred():
                    nc.vector.copy_predicated(st["tanh_sb"], mknat_t, negt)
                    st["sacc"] = sm.tile([128, 4], f32, tag="sacc",
                                         name="sacc")
                    st["scratch"] = sm.tile([128, N], f32, tag="scratch",
                                            name="scratch")

                def s_exp(i):
                    nc.scalar.activation(
                        st["scratch"], st["tanh_sb"][:, i, :], AF.Exp,
                        scale=10.0, accum_out=st["sacc"][:, i:i + 1])

                def s_z2():
                    pst = ptiny.tile([4, 128], f32, tag="tiny", name="pst")
                    nc.tensor.transpose(pst, st["sacc"], W["ident"])
                    ssb = sm.tile([4, 128], f32, tag="ssb", name="ssb")
                    nc.vector.tensor_copy(ssb, pst)
                    z2 = sm.tile([4, 32], f32, tag="z2", name="z2")
                    nc.vector.tensor_reduce(
                        z2, ssb.rearrange("p (t c) -> p t c", c=4),
                        axis=mybir.AxisListType.X, op=OP.add)
                    st["z2"] = z2

                def s_ln1():
                    zi = st["z2"].bitcast(i32)
                    ei = sm.tile([4, 32], i32, tag="ei", name="ei")
                    nc.vector.tensor_scalar(ei, zi, 23, None,
                                            OP.logical_shift_right)
                    ef = sm.tile([4, 32], f32, tag="ef", name="ef")
                    nc.vector.tensor_copy(ef, ei)
                    mi = sm.tile([4, 32], i32, tag="mi", name="mi")
                    nc.vector.tensor_scalar(mi, zi, 0x7FFFFF, 0x3F800000,
                                            OP.bitwise_and, OP.bitwise_or)
                    acc = sm.tile([4, 32], f32, tag="lnacc", name="acc")
                    nc.vector.tensor_scalar(acc, mi.bitcast(f32), LN_COEF[7],
                                            LN_COEF[6], OP.mult, OP.add)
                    st["ef"], st["mi"], st["acc"] = ef, mi, acc

                def s_ln2(ks):
                    mf = st["mi"].bitcast(f32)
                    for k in ks:
                        nc.vector.tensor_tensor(st["acc"], st["acc"], mf,
                                                OP.mult)
                        nc.vector.tensor_scalar_add(st["acc"], st["acc"],
                                                    LN_COEF[k])

                def s_ln3():
                    nc.vector.tensor_scalar(st["ef"], st["ef"], LN2,
                                            -127.0 * LN2, OP.mult, OP.add)
                    nc.vector.tensor_tensor(st["acc"], st["acc"], st["ef"],
                                            OP.add)

                def s_bias():
                    pzt = ptiny.tile([32, 4], f32, tag="tiny", name="pzt")
                    nc.tensor.transpose(pzt, st["acc"], W["ident"][:4, :4])
                    lzt = sm.tile([32, 4], f32, tag="lzt", name="lzt")
                    nc.vector.tensor_copy(lzt, pzt)
                    pbias = ptiny.tile([128, 4], f32, tag="tiny", name="pbias")
                    nc.tensor.matmul(pbias, lhsT=W["p432"], rhs=lzt)
                    bias = sm.tile([128, 4], f32, tag="bias", name="bias")
                    nc.vector.tensor_copy(bias, pbias)
                    st["bias"] = bias

                def s_out(i):
                    if i == 0:
                        st["out_sb"] = big.tile([128, C, N], f32, tag="outsb",
                                                name="out_sb")
                    nc.vector.tensor_scalar(
                        st["out_sb"][:, i, :], st["tanh_sb"][:, i, :], 10.0,
                        st["bias"][:, i:i + 1], OP.mult, OP.subtract)
                    if i == 3:
                        nc.sync.dma_start(out=out[:, b, :, :],
                                          in_=st["out_sb"])

                return [
                    s_recip,
                    lambda: s_un(0), lambda: s_un(1),
                    s_q2,
                    lambda: s_tanh(0), lambda: s_tanh(1),
                    s_pred,
                    lambda: s_exp(0), lambda: s_exp(1),
                    lambda: s_exp(2), lambda: s_exp(3),
                    s_z2, s_ln1,
                    lambda: s_ln2([5, 4, 3]), lambda: s_ln2([2, 1, 0]),
                    s_ln3, s_bias,
                    lambda: s_out(0), lambda: s_out(1),
                    lambda: s_out(2), lambda: s_out(3),
                ]

            # prologue of batch 0 runs up front
            cur = {}
            for step in build_prologue(0, cur):
                step()

            for b in range(nb):
                nxt = {}
                if b + 1 < nb:
                    pend_pro = build_prologue(b + 1, nxt)

                # ---------- attention passes ----------
                zsb = sm.tile([4, 1024], f32, tag="zsb")
                usb = {}
                psu = {}
                vaugs = (cur["vauga"], cur["vaugb"])
                mkb_t = cur["mkb"]

                def issue_u(pi, j, esA, esB):
                    if j == 0:
                        psu[pi] = pacc.tile([128, Q], f32, tag="u",
                                            name=f"psu{pi}")
                    # 4 concurrent col-tiled U MMs (accumulate over j)
                    for g in range(4):
                        es = esA if g < 2 else esB
                        nc.tensor.matmul(
                            psu[pi][32 * g:32 * g + 32, :],
                            lhsT=vaugs[pi][:, j, 32 * g:32 * g + 32],
                            rhs=es[:, g % 2, :],
                            start=(j == 0), stop=(j == 3),
                            tile_position=(0, 32 * g),
                            skip_group_check=True)
                    if j == 3:
                        u_sb = wk.tile([128, Q], bf16, tag="usb")
                        nc.vector.tensor_copy(u_sb, psu[pi])
                        usb[pi] = u_sb
                        pz = ptiny.tile([4, Q], f32, tag="tiny")
                        nc.tensor.matmul(pz, lhsT=W["g16"], rhs=u_sb)
                        nc.vector.tensor_copy(zsb[:, Q * pi:Q * (pi + 1)], pz)

                prev = None
                for pi, (k1t, q1t_sb) in enumerate(
                        ((cur["k1ta"], cur["q1ta"]), (cur["k1tb"], cur["q1tb"]))):
                    for j in range(4):
                        # 4 concurrent row-tiled score MMs (strips 0..3)
                        pssA = pscore.tile([128, 2, Q], f32, tag="score")
                        pssB = pscore.tile([128, 2, Q], f32, tag="score")
                        for g in range(4):
                            pss = pssA if g < 2 else pssB
                            # K=32 (rows 16..31 of each strip are zero by the
                            # head-perm layout) - full 32-row tiles overlap
                            # better than K=16 slices
                            sl = slice(32 * g, 32 * g + 32)
                            nc.tensor.matmul(
                                pss[:, g % 2, :],
                                lhsT=k1t[sl, 128 * j:128 * (j + 1)],
                                rhs=q1t_sb[sl, :],
                                tile_position=(32 * g, 0))
                        # exp -> bf16, then keep-mask multiply on DVE
                        esA = esp.tile([128, 2, Q], bf16, tag="esA")
                        esB = esp.tile([128, 2, Q], bf16, tag="esB")
                        mk_b = mkb_t[:, j:j + 1, :].broadcast_to([128, 2, Q])
                        nc.scalar.activation(esA, pssA, AF.Exp)
                        nc.vector.tensor_tensor(esA, esA, mk_b, OP.mult)
                        nc.scalar.activation(esB, pssB, AF.Exp)
                        nc.vector.tensor_tensor(esB, esB, mk_b, OP.mult)
                        if prev is not None:
                            issue_u(*prev)
                        prev = (pi, j, esA, esB)
                        drain_epi(2)
                        drain_pro(1)
                issue_u(*prev)
                drain_epi(99)
                drain_pro(99)

                pend_epi = build_epilogue(b, usb, zsb, cur["k2t"],
                                          cur["mknat"])
                cur = nxt

            while pend_epi:
                pend_epi.pop(0)()

    nc.compile()
    return nc


_CACHED = None


def _get_nc():
    global _CACHED
    if _CACHED is None:
        _CACHED = build_kernel()
    return _CACHED


def kernel(**inputs):
    from concourse.bass_utils import run_bass_kernel_spmd

    core_ins = _host_prep(inputs)
    nc = _get_nc()
    res = run_bass_kernel_spmd(nc, core_ins, core_ids=list(range(NCORES)))
    outs = [_unscramble(r["out"]) for r in res.results]   # each [T, NB, 2048]
    return np.concatenate(outs, axis=1)                   # [T, B, 2048]


def _unscramble(dev):
    """Device [128 q'=(t',c), nb, 4 i, 512 n] -> [T, nb, C*N] with t=32i+t'."""
    nb = dev.shape[1]
    return (dev.reshape(32, C, nb, 4, N)
            .transpose(3, 0, 2, 1, 4)
            .reshape(T, nb, C * N))


# revision 45
# speedup vs baseline: 1.0848x; 1.0468x over previous
"""Trainium2 Bass kernel for nn_DecoderCell (B=128,N=512,C=4,T=128,D=128,H=8).

Strategy: pure data-parallel over batch B across 8 NeuronCores (16 b/core).
Per batch, attention scores are computed transposed ([n, q] layout, q=(t,c))
with per-head K=16 matmuls packed 4-at-a-time onto distinct PE row groups so
they execute concurrently; the boolean mask is applied as a keep-mask (0/1)
multiply on the Vector engine after the exp (exp(s+NEG*m) == exp(s)*keep),
which keeps full-array identity matmuls off the PE. U=V@expS uses 4
concurrent column-tiled matmuls. Softmax runs unnormalized (exp on ScalarE,
denominators via an augmented-V ones column), and the final log-softmax uses
ACT accum_out row sums plus a DVE bit-twiddle ln. Matmul operands are bf16
(PSUM accumulation in f32).
"""
import numpy as np
import ml_dtypes

D = 128
N = 512
C = 4
T = 128
Q = T * C          # 512 queries per batch, q = t*C + c
H = 8
DH = 16
NB = 16            # batches per core
NCORES = 8
NEG = -1e9         # reference mask value
NEGT = -1e8        # written into tanh slots; ×10 → -1e9

# ln(m) on [1,2), power-series coeffs (deg 7, max err 5.6e-7)
LN_COEF = [
    -2.242481818575902, 4.911042808776086, -5.126667255647402,
    3.932633388234101, -2.0202020938525127, 0.6590148821953554,
    -0.12345843186141042, 0.010119082927599773,
]
LN2 = 0.6931471805599453

HA = [0, 1, 2, 3]
HB = [4, 5, 6, 7]
BF = ml_dtypes.bfloat16
F8 = ml_dtypes.float8_e4m3


def _perm_cols(W, heads):
    """Columns of W[*,128] so head g sits at cols 32g..32g+15, zeros after."""
    out = np.zeros_like(W)
    for g, h in enumerate(heads):
        out[:, 32 * g:32 * g + 16] = W[:, 16 * h:16 * h + 16]
    return out


def _perm_rows(W, heads):
    out = np.zeros_like(W)
    for g, h in enumerate(heads):
        out[32 * g:32 * g + 16, :] = W[16 * h:16 * h + 16, :]
    return out


def _host_prep(inputs):
    """Full-input numpy prep -> per-core input dicts."""
    ne = np.ascontiguousarray(inputs["node_embeddings"], np.float32)
    ge = np.ascontiguousarray(inputs["graph_embedding"], np.float32)
    sc = np.ascontiguousarray(inputs["step_context"], np.float32)
    mask = np.asarray(inputs["mask"])
    B = ne.shape[0]

    net = ne.transpose(0, 2, 1).astype(BF)                        # [B,D,N]
    scq = sc[:, :, :, 0, :].transpose(1, 3, 0, 2).reshape(B, D + 1, Q)
    scm = scq[:, :D, :].astype(BF)
    scl = scq[:, D:, :].astype(BF)
    m4 = mask[:, :, :, 0, :]                                      # [T,B,C,N] bool
    # keep mask (1 = unmasked) transposed [B, N, Q] bf16
    mkb = (1.0 - m4.transpose(1, 3, 0, 2).reshape(B, N, Q)
           .astype(np.float32)).astype(BF)
    mknat = m4.transpose(1, 0, 2, 3).reshape(B, Q, N).astype(np.uint8)
    gt = np.ascontiguousarray(ge.T).astype(BF)                    # [D,B]

    s = np.float32(1.0 / np.sqrt(DH))
    Wk1 = np.asarray(inputs["Wk1"], np.float32)
    Wqs = np.asarray(inputs["Wq_step"], np.float32) * s
    Wqf = np.asarray(inputs["Wq_fixed"], np.float32) * s
    wqsa, wqsb = _perm_cols(Wqs, HA), _perm_cols(Wqs, HB)
    # graph-embedding term precomputed on host: qfix[b] = ge[b] @ Wq_fixed*s,
    # then head-permuted; packed with the D+1-th context row's weights into a
    # rank-2 matmul rhs [scl; ones]
    qfix = ge.astype(np.float32) @ Wqf                      # [B, D]
    qfa = _perm_cols(qfix, HA)                              # [B, D] per-pass
    qfb = _perm_cols(qfix, HB)
    bfw = lambda x: np.ascontiguousarray(x).astype(BF)
    wqsal, wqsbl = wqsa[D], wqsb[D]                         # [D] rank-1 rows
    weights = {
        "wk1a": bfw(_perm_cols(Wk1, HA)), "wk1b": bfw(_perm_cols(Wk1, HB)),
        "wqsam": bfw(wqsa[:D]),
        "wqsbm": bfw(wqsb[:D]),
        "wv": bfw(inputs["Wv"]),
        "wk2s": bfw(np.asarray(inputs["Wk2"], np.float32)
                    / np.float32(np.sqrt(D))),
        "wouta": bfw(_perm_rows(np.asarray(inputs["Wout"], np.float32), HA)),
        "woutb": bfw(_perm_rows(np.asarray(inputs["Wout"], np.float32), HB)),
        "ident": np.eye(128, dtype=np.float32),
        # e4[g, 32g+r] = 1 for r<16 (normalization broadcast)
        "e4": np.stack([
            np.concatenate([np.zeros(32 * g, np.float32),
                            np.ones(16, np.float32),
                            np.zeros(128 - 32 * g - 16, np.float32)])
            for g in range(4)]).astype(BF),
        # g16[32g+16, g] = 1 (Z row gather)
        "g16": np.stack([
            (np.arange(128) == 32 * g + 16).astype(np.float32)
            for g in range(4)], axis=1).astype(BF),
        # p432[tp, 4tp+c] = 1 (lnZ partition broadcast)
        "p432": np.stack([
            ((np.arange(128) // 4) == tp).astype(np.float32)
            for tp in range(32)]),
    }

    core_ins = []
    for ci in range(NCORES):
        b0 = ci * NB
        sl = slice(b0, b0 + NB)
        m = dict(weights)
        m.update({
            "net": np.ascontiguousarray(net[sl]),
            "scm": np.ascontiguousarray(scm[sl]),
            "scl2": np.ascontiguousarray(np.stack(
                [scl[sl][:, 0, :], np.ones_like(scl[sl][:, 0, :])],
                axis=1)).astype(BF),
            "qf2a": np.ascontiguousarray(np.stack(
                [np.broadcast_to(wqsal, (NB, D)),
                 qfa[sl]], axis=1)).astype(BF),
            "qf2b": np.ascontiguousarray(np.stack(
                [np.broadcast_to(wqsbl, (NB, D)),
                 qfb[sl]], axis=1)).astype(BF),
            "mkb": np.ascontiguousarray(mkb[sl]),
            "mknat": np.ascontiguousarray(mknat[sl]),
        })
        core_ins.append(m)
    return core_ins


def build_kernel(nb=NB):
    import concourse.bacc as bacc
    import concourse.mybir as mybir
    import concourse.tile as tile

    dt = mybir.dt
    f32, bf16, u8, i32 = dt.float32, dt.bfloat16, dt.uint8, dt.int32
    AF = mybir.ActivationFunctionType
    OP = mybir.AluOpType

    nc = bacc.Bacc("TRN2", target_bir_lowering=False, debug=False,
                   num_devices=NCORES)

    din = {}
    def dram(name, shape, dtype, kind="ExternalInput"):
        din[name] = nc.dram_tensor(name, shape, dtype, kind=kind)
        return din[name]

    net = dram("net", [nb, D, N], bf16)
    scm = dram("scm", [nb, D, Q], bf16)
    scl2 = dram("scl2", [nb, 2, Q], bf16)
    qf2a = dram("qf2a", [nb, 2, D], bf16)
    qf2b = dram("qf2b", [nb, 2, D], bf16)
    mkb = dram("mkb", [nb, N, Q], bf16)
    mknat = dram("mknat", [nb, Q, N], u8)
    for w in ("wk1a", "wk1b", "wqsam", "wqsbm", "wv", "wk2s",
              "wouta", "woutb", "e4", "g16"):
        shape = [4, 128] if w == "e4" else ([128, 4] if w == "g16" else [128, 128])
        dram(w, shape, bf16)
    dram("ident", [128, 128], f32)
    dram("p432", [32, 128], f32)
    # device layout [q'=(t', c), b, i, n]; host reassembles t = 32*i + t'
    out = dram("out", [128, nb, 4, N], f32, kind="ExternalOutput")

    with tile.TileContext(nc) as tc:
        from contextlib import ExitStack
        with ExitStack() as ctx:
            wp = ctx.enter_context(tc.tile_pool(name="wp", bufs=1))
            io = ctx.enter_context(tc.tile_pool(name="io", bufs=2))
            wk = ctx.enter_context(tc.tile_pool(name="wk", bufs=2))
            esp = ctx.enter_context(tc.tile_pool(name="esp", bufs=4))
            big = ctx.enter_context(tc.tile_pool(name="big", bufs=2))
            sm = ctx.enter_context(tc.tile_pool(name="sm", bufs=2))
            # PSUM: 8 banks total.
            # pscore 2x[128,2,512]=4, pacc 1x[128,512]=1, pmisc 2x[128,512]=2,
            # ptiny 1x[*,<=512]=1
            pscore = ctx.enter_context(
                tc.tile_pool(name="pscore", bufs=2, space="PSUM"))
            pacc = ctx.enter_context(
                tc.tile_pool(name="pacc", bufs=1, space="PSUM"))
            pmisc = ctx.enter_context(
                tc.tile_pool(name="pmisc", bufs=2, space="PSUM"))
            ptiny = ctx.enter_context(
                tc.tile_pool(name="ptiny", bufs=1, space="PSUM"))

            # --- static weights/constants to SBUF ---
            W = {}
            for wn in ("wk1a", "wk1b", "wqsam", "wqsbm", "wv",
                       "wk2s", "wouta", "woutb", "ident", "e4", "g16",
                       "p432"):
                t = wp.tile(list(din[wn].shape), din[wn].dtype, tag=f"w_{wn}")
                nc.sync.dma_start(out=t, in_=din[wn][:, :])
                W[wn] = t
            negt = wp.tile([128, C, N], f32, tag="negt")
            nc.gpsimd.memset(negt, NEGT)

            pend_epi = []
            pend_pro = []

            def drain_epi(n=1):
                for _ in range(min(n, len(pend_epi))):
                    pend_epi.pop(0)()

            def drain_pro(n=1):
                for _ in range(min(n, len(pend_pro))):
                    pend_pro.pop(0)()

            def build_prologue(b, dst):
                """Closure list loading batch b's inputs and computing its
                projections into `dst`; drained during batch b-1's attention
                so the serial projection chain overlaps useful work."""

                def p_dma():
                    net_t = io.tile([D, N], bf16, tag="net", name="net_t")
                    nc.sync.dma_start(out=net_t, in_=net[b])
                    scm_t = io.tile([D, Q], bf16, tag="scm", name="scm_t")
                    nc.sync.dma_start(out=scm_t, in_=scm[b])
                    scl_t = io.tile([2, Q], bf16, tag="scl", name="scl_t")
                    nc.sync.dma_start(out=scl_t, in_=scl2[b])
                    qfa_t = io.tile([2, D], bf16, tag="qfa", name="qfa_t")
                    nc.sync.dma_start(out=qfa_t, in_=qf2a[b])
                    qfb_t = io.tile([2, D], bf16, tag="qfb", name="qfb_t")
                    nc.sync.dma_start(out=qfb_t, in_=qf2b[b])
                    mkb_t = io.tile([128, 4, Q], bf16, tag="mkb",
                                    name="mkb_t")
                    nc.sync.dma_start(
                        out=mkb_t,
                        in_=mkb[b].rearrange("(j p) q -> p j q", p=128))
                    mknat_t = io.tile([128, 4, N], u8, tag="mknat",
                                      name="mknat_t")
                    nc.sync.dma_start(
                        out=mknat_t,
                        in_=mknat[b].rearrange("(i p) n -> p i n", p=128))
                    dst["net"], dst["scm"], dst["scl"] = net_t, scm_t, scl_t
                    dst["qfa"], dst["qfb"] = qfa_t, qfb_t
                    dst["mkb"], dst["mknat"] = mkb_t, mknat_t

                def p_proj(wn, tag, eng):
                    ps = pmisc.tile([128, N], f32, tag="proj", name="ps")
                    nc.tensor.matmul(ps, lhsT=W[wn], rhs=dst["net"])
                    sb = wk.tile([128, N], bf16, tag=tag, name=tag)
                    if eng == "s":
                        nc.scalar.copy(sb, ps)
                    else:
                        nc.vector.tensor_copy(sb, ps)
                    dst[tag] = sb

                def p_vaug_init():
                    vauga = wk.tile([128, 4, 128], bf16, tag="vauga",
                                    name="vauga")
                    vaugb = wk.tile([128, 4, 128], bf16, tag="vaugb",
                                    name="vaugb")
                    for va in (vauga, vaugb):
                        nc.gpsimd.memset(va, 0.0)
                        nc.gpsimd.memset(
                            va.rearrange("p j (g r) -> p j g r",
                                         g=4)[:, :, :, 16:17], 1.0)
                    dst["vauga"], dst["vaugb"] = vauga, vaugb

                def p_v(j):
                    pv = pmisc.tile([128, 128], f32, tag="proj", name="pv")
                    nc.tensor.matmul(
                        pv[:, :128], lhsT=dst["net"][:, 128 * j:128 * (j + 1)],
                        rhs=W["wv"])
                    for va, c0 in ((dst["vauga"], 0), (dst["vaugb"], 64)):
                        nc.vector.tensor_copy(
                            va[:, j, :].rearrange("p (g r) -> p g r",
                                                  g=4)[:, :, 0:16],
                            pv[:, c0:c0 + 64].rearrange("p (g r) -> p g r",
                                                        g=4))

                def p_q1(wm, qf, tag):
                    ps = pmisc.tile([128, Q], f32, tag="proj", name="ps")
                    nc.tensor.matmul(ps, lhsT=W[wm], rhs=dst["scm"],
                                     start=True, stop=False)
                    nc.tensor.matmul(ps, lhsT=dst[qf], rhs=dst["scl"],
                                     start=False, stop=True)
                    sb = wk.tile([128, Q], bf16, tag=tag, name=tag)
                    nc.scalar.copy(sb, ps)
                    dst[tag] = sb

                return [
                    p_dma,
                    lambda: p_proj("wk1a", "k1ta", "v"),
                    lambda: p_proj("wk1b", "k1tb", "v"),
                    lambda: p_proj("wk2s", "k2t", "s"),
                    p_vaug_init,
                    lambda: p_v(0), lambda: p_v(1),
                    lambda: p_v(2), lambda: p_v(3),
                    lambda: p_q1("wqsam", "qfa", "q1ta"),
                    lambda: p_q1("wqsbm", "qfb", "q1tb"),
                ]

            def build_epilogue(b, usb, zsb, k2t, mknat_t):
                """Closure list for batch b's post-attention chain; emitted
                interleaved into batch b+1's attention to avoid head-of-line
                stalls on the in-order engine queues."""
                st = {}

                def s_recip():
                    rinv = sm.tile([4, 1024], f32, tag="rinv", name="rinv")
                    nc.vector.reciprocal_approx_fast(out=rinv, in_=zsb)
                    rinvb = sm.tile([4, 1024], bf16, tag="rinvb", name="rinvb")
                    nc.vector.tensor_copy(rinvb, rinv)
                    st["rinvb"] = rinvb

                def s_un(pi):
                    pbc = pmisc.tile([128, Q], f32, tag="proj", name="pbc")
                    nc.tensor.matmul(
                        pbc, lhsT=W["e4"],
                        rhs=st["rinvb"][:, Q * pi:Q * (pi + 1)])
                    u_n = wk.tile([128, Q], bf16, tag=f"un{pi}", name="u_n")
                    nc.vector.tensor_tensor(u_n, usb[pi], pbc, OP.mult)
                    st[f"un{pi}"] = u_n

                def s_q2():
                    pq2 = pmisc.tile([128, Q], f32, tag="proj", name="pq2")
                    nc.tensor.matmul(pq2, lhsT=W["wouta"], rhs=st["un0"],
                                     start=True, stop=False)
                    nc.tensor.matmul(pq2, lhsT=W["woutb"], rhs=st["un1"],
                                     start=False, stop=True)
                    q2t = wk.tile([128, Q], bf16, tag="q2t", name="q2t")
                    nc.vector.tensor_copy(q2t, pq2)
                    st["q2t"] = q2t

                def s_tanh(ii):
                    if ii == 0:
                        st["tanh_sb"] = big.tile([128, C, N], f32, tag="tanh",
                                                 name="tanh_sb")
                    pl = pscore.tile([128, 2, N], f32, tag="score", name="pl")
                    for i2 in range(2):
                        i = 2 * ii + i2
                        nc.tensor.matmul(
                            pl[:, i2, :],
                            lhsT=st["q2t"][:, 128 * i:128 * (i + 1)], rhs=k2t)
                    nc.scalar.activation(
                        st["tanh_sb"][:, 2 * ii:2 * ii + 2, :], pl, AF.Tanh)

                def s_pred():
                    nc.vector.copy_predicated(st["tanh_sb"], mknat_t, negt)
                    st["sacc"] = sm.tile([128, 4], f32, tag="sacc",
                                         name="sacc")
                    st["scratch"] = sm.tile([128, N], f32, tag="scratch",
                                            name="scratch")

                def s_exp(i):
                    nc.scalar.activation(
                        st["scratch"], st["tanh_sb"][:, i, :], AF.Exp,
                        scale=10.0, accum_out=st["sacc"][:, i:i + 1])

                def s_z2():
                    pst = ptiny.tile([4, 128], f32, tag="tiny", name="pst")
                    nc.tensor.transpose(pst, st["sacc"], W["ident"])
                    ssb = sm.tile([4, 128], f32, tag="ssb", name="ssb")
                    nc.vector.tensor_copy(ssb, pst)
                    z2 = sm.tile([4, 32], f32, tag="z2", name="z2")
                    nc.vector.tensor_reduce(
                        z2, ssb.rearrange("p (t c) -> p t c", c=4),
                        axis=mybir.AxisListType.X, op=OP.add)
                    st["z2"] = z2

                def s_ln1():
                    zi = st["z2"].bitcast(i32)
                    ei = sm.tile([4, 32], i32, tag="ei", name="ei")
                    nc.vector.tensor_scalar(ei, zi, 23, None,
                                            OP.logical_shift_right)
                    ef = sm.tile([4, 32], f32, tag="ef", name="ef")
                    nc.vector.tensor_copy(ef, ei)
                    mi = sm.tile([4, 32], i32, tag="mi", name="mi")
                    nc.vector.tensor_scalar(mi, zi, 0x7FFFFF, 0x3F800000,
                                            OP.bitwise_and, OP.bitwise_or)
                    acc = sm.tile([4, 32], f32, tag="lnacc", name="acc")
                    nc.vector.tensor_scalar(acc, mi.bitcast(f32), LN_COEF[7],
                                            LN_COEF[6], OP.mult, OP.add)
                    st["ef"], st["mi"], st["acc"] = ef, mi, acc

                def s_ln2(ks):
                    mf = st["mi"].bitcast(f32)
                    for k in ks:
                        nc.vector.tensor_tensor(st["acc"], st["acc"], mf,
                                                OP.mult)
                        nc.vector.tensor_scalar_add(st["acc"], st["acc"],
                                                    LN_COEF[k])

                def s_ln3():
                    nc.vector.tensor_scalar(st["ef"], st["ef"], LN2,
                                            -127.0 * LN2, OP.mult, OP.add)
                    nc.vector.tensor_tensor(st["acc"], st["acc"], st["ef"],
                                            OP.add)

                def s_bias():
                    pzt = ptiny.tile([32, 4], f32, tag="tiny", name="pzt")
                    nc.tensor.transpose(pzt, st["acc"], W["ident"][:4, :4])
                    lzt = sm.tile([32, 4], f32, tag="lzt", name="lzt")
                    nc.vector.tensor_copy(lzt, pzt)
                    pbias = ptiny.tile([128, 4], f32, tag="tiny", name="pbias")
                    nc.tensor.matmul(pbias, lhsT=W["p432"], rhs=lzt)
                    bias = sm.tile([128, 4], f32, tag="bias", name="bias")
                    nc.vector.tensor_copy(bias, pbias)
                    st["bias"] = bias

                def s_out(i):
                    if i == 0:
                        st["out_sb"] = big.tile([128, C, N], f32, tag="outsb",
                                                name="out_sb")
                    nc.vector.tensor_scalar(
                        st["out_sb"][:, i, :], st["tanh_sb"][:, i, :], 10.0,
                        st["bias"][:, i:i + 1], OP.mult, OP.subtract)
                    if i == 3:
                        nc.sync.dma_start(out=out[:, b, :, :],
                                          in_=st["out_sb"])

                return [
                    s_recip,
                    lambda: s_un(0), lambda: s_un(1),
                    s_q2,
                    lambda: s_tanh(0), lambda: s_tanh(1),
                    s_pred,
                    lambda: s_exp(0), lambda: s_exp(1),
                    lambda: s_exp(2), lambda: s_exp(3),
                    s_z2, s_ln1,
                    lambda: s_ln2([5, 4, 3]), lambda: s_ln2([2, 1, 0]),
                    s_ln3, s_bias,
                    lambda: s_out(0), lambda: s_out(1),
                    lambda: s_out(2), lambda: s_out(3),
                ]

            # prologue of batch 0 runs up front
            cur = {}
            for step in build_prologue(0, cur):
                step()

            for b in range(nb):
                nxt = {}
                if b + 1 < nb:
                    pend_pro = build_prologue(b + 1, nxt)

                # ---------- attention passes ----------
                zsb = sm.tile([4, 1024], f32, tag="zsb")
                usb = {}
                psu = {}
                vaugs = (cur["vauga"], cur["vaugb"])
                mkb_t = cur["mkb"]

                def issue_u(pi, j, esA, esB):
                    if j == 0:
                        psu[pi] = pacc.tile([128, Q], f32, tag="u",
                                            name=f"psu{pi}")
                    # 4 concurrent col-tiled U MMs (accumulate over j)
                    for g in range(4):
                        es = esA if g < 2 else esB
                        nc.tensor.matmul(
                            psu[pi][32 * g:32 * g + 32, :],
                            lhsT=vaugs[pi][:, j, 32 * g:32 * g + 32],
                            rhs=es[:, g % 2, :],
                            start=(j == 0), stop=(j == 3),
                            tile_position=(0, 32 * g),
                            skip_group_check=True)
                    if j == 3:
                        u_sb = wk.tile([128, Q], bf16, tag="usb")
                        nc.vector.tensor_copy(u_sb, psu[pi])
                        usb[pi] = u_sb
                        pz = ptiny.tile([4, Q], f32, tag="tiny")
                        nc.tensor.matmul(pz, lhsT=W["g16"], rhs=u_sb)
                        nc.vector.tensor_copy(zsb[:, Q * pi:Q * (pi + 1)], pz)

                prev = None
                for pi, (k1t, q1t_sb) in enumerate(
                        ((cur["k1ta"], cur["q1ta"]), (cur["k1tb"], cur["q1tb"]))):
                    for j in range(4):
                        # 4 concurrent row-tiled score MMs (strips 0..3)
                        pssA = pscore.tile([128, 2, Q], f32, tag="score")
                        pssB = pscore.tile([128, 2, Q], f32, tag="score")
                        for g in range(4):
                            pss = pssA if g < 2 else pssB
                            # K=32 (rows 16..31 of each strip are zero by the
                            # head-perm layout) - full 32-row tiles overlap
                            # better than K=16 slices
                            sl = slice(32 * g, 32 * g + 32)
                            nc.tensor.matmul(
                                pss[:, g % 2, :],
                                lhsT=k1t[sl, 128 * j:128 * (j + 1)],
                                rhs=q1t_sb[sl, :],
                                tile_position=(32 * g, 0))
                        # exp -> bf16, then keep-mask multiply on DVE
                        esA = esp.tile([128, 2, Q], bf16, tag="esA")
                        esB = esp.tile([128, 2, Q], bf16, tag="esB")
                        mk_b = mkb_t[:, j:j + 1, :].broadcast_to([128, 2, Q])
                        nc.scalar.activation(esA, pssA, AF.Exp)
                        nc.vector.tensor_tensor(esA, esA, mk_b, OP.mult)
                        nc.scalar.activation(esB, pssB, AF.Exp)
                        nc.vector.tensor_tensor(esB, esB, mk_b, OP.mult)
                        if prev is not None:
                            issue_u(*prev)
                        prev = (pi, j, esA, esB)
                        drain_epi(1)
                        drain_pro(1)
                issue_u(*prev)
                drain_epi(99)
                drain_pro(99)

                pend_epi = build_epilogue(b, usb, zsb, cur["k2t"],
                                          cur["mknat"])
                cur = nxt

            while pend_epi:
                pend_epi.pop(0)()

    nc.compile()
    return nc


_CACHED = None


def _get_nc():
    global _CACHED
    if _CACHED is None:
        _CACHED = build_kernel()
    return _CACHED


def kernel(**inputs):
    from concourse.bass_utils import run_bass_kernel_spmd

    core_ins = _host_prep(inputs)
    nc = _get_nc()
    res = run_bass_kernel_spmd(nc, core_ins, core_ids=list(range(NCORES)))
    outs = [_unscramble(r["out"]) for r in res.results]   # each [T, NB, 2048]
    return np.concatenate(outs, axis=1)                   # [T, B, 2048]


def _unscramble(dev):
    """Device [128 q'=(t',c), nb, 4 i, 512 n] -> [T, nb, C*N] with t=32i+t'."""
    nb = dev.shape[1]
    return (dev.reshape(32, C, nb, 4, N)
            .transpose(3, 0, 2, 1, 4)
            .reshape(T, nb, C * N))
